# revision 31
# baseline (speedup 1.0000x reference)
"""Gemma3 sliding-window attention on 8 Trainium2 NeuronCores — bf16 pipeline.

Sharding: core c handles batch b=c//4 and head-group g=c%4 (4 of 16 q heads,
2 of 8 kv heads). wq/wk/wv column-split, wo row-split; the 4 partial outputs
per batch are summed on host.

All matmul operands are bf16 (PSUM accumulation stays fp32). The kernel is a
single software pipeline over 256-column sequence chunks: attention for chunk
qc-1 and its output projection are emitted between the projections of chunk
qc-1 and qc, so PE matmuls, Act exps, and DVE/Pool elementwise work overlap
instead of running as three serial phases.

Projections run two heads at a time into [128,512] PSUM tiles (PSUM is
allocated in whole 2KB banks; this also halves Act/DVE instruction count).
RMS-norm rsqrt is computed as exp(-0.5*ln(var+eps)) so every activation in
the kernel (square/ln/exp) lives in one activation-function table set — no
mid-kernel table reloads. Attention works on transposed tiles ([k, q],
d-on-partition q/k) so softmax denominators come from a ones-matmul over a
DVE-accumulated prob-tile sum, and sliding-window edge/diagonal tiles are
emitted as 128-wide segments so no fully-masked work is done.

Host-side tensors are pre-shuffled to partition-major layouts so each logical
transfer is one large DMA (the HWDGE queue costs ~625ns per DMA instruction
regardless of size): hidden-state chunks, packed rope tables, each weight
matrix, and 4-row-group output tiles are single DMAs.
"""

import math
import numpy as np
import ml_dtypes

import concourse.bacc as bacc
import concourse.mybir as mybir
import concourse.tile as tile
from concourse.bass_utils import run_bass_kernel_spmd

dt = mybir.dt
AFT = mybir.ActivationFunctionType
ALU = mybir.AluOpType

B, S, H = 2, 2048, 2048
NQ, NKV, D = 16, 8, 128          # global heads
NQC, NKVC = 4, 2                 # heads per core
WIN = 1024
EPS = 1e-6
THETA = 10000.0
P = 128
SCP = 256                        # seq chunk
SCO = 512                        # output-projection column chunk
NHT = H // P                     # 16 hidden tiles
NST = S // P                     # 16 seq tiles
NCH = S // SCP                   # 8 chunks
WT = WIN // P                    # 8 window tiles

_CACHE = {}


def _segments(qc):
    """Score/PV segments for q-chunk qc: (t, c0, w, mask) with chunk-local
    q columns [c0, c0+w); mask in {None,'dm0','dm','em','em1'}."""
    u0, u1 = 2 * qc, 2 * qc + 1
    segs = []
    for t in range(max(0, u0 - WT), u1 + 1):
        dd0, dd1 = u0 - t, u1 - t
        if dd0 == WT:
            segs.append((t, 0, P, "em"))        # edge for subtile 0
        elif dd1 == WT:
            segs.append((t, 0, 2 * P, "em1"))   # full + edge-masked half B
        elif dd0 == 0:
            segs.append((t, 0, 2 * P, "dm0"))   # diag-masked half A + full
        elif dd1 == 0:
            segs.append((t, P, P, "dm"))        # diag for subtile 1
        else:
            segs.append((t, 0, 2 * P, None))
    return segs


def _pv_order(segs):
    """Order for PV/den accumulation: one 256-wide seg first. Only the first
    matmul carries start=True — it marks the whole 2KB PSUM row pending-zero,
    and later matmuls auto-zero any still-pending bytes they write (a second
    start=True would discard earlier segments' accumulation)."""
    narrow = [s for s in segs if s[2] == P]
    wide = [s for s in segs if s[2] == 2 * P]
    lead = next((s for s in wide if s[3] is None), wide[0])
    rest = [s for s in wide if s is not lead]
    return [lead] + narrow + rest


def _build_nc():
    if "nc" in _CACHE:
        return _CACHE["nc"]
    nc = bacc.Bacc("TRN2", target_bir_lowering=False, debug=False, num_devices=8)
    f32, bf = dt.float32, dt.bfloat16

    # p-major host layouts: [...] dims are (partition, block, inner)
    hsd = nc.dram_tensor("hsd", [P, NCH, NHT * SCP], bf, kind="ExternalInput").ap()
    wqd = nc.dram_tensor("wqd", [P, NHT * NQC * D], bf, kind="ExternalInput").ap()
    wkd = nc.dram_tensor("wkd", [P, NHT * NKVC * D], bf, kind="ExternalInput").ap()
    wvd = nc.dram_tensor("wvd", [P, NHT * NKVC * D], bf, kind="ExternalInput").ap()
    wod = nc.dram_tensor("wod", [P, NQC * H], bf, kind="ExternalInput").ap()
    tabd = nc.dram_tensor("tabd", [P, NCH, 4 * SCP], bf, kind="ExternalInput").ap()
    cstd = nc.dram_tensor("cstd", [P, 6 * P], bf, kind="ExternalInput").ap()
    yTd = nc.dram_tensor("yT", [P, NHT, S], bf, kind="ExternalOutput").ap()

    with tile.TileContext(nc) as tc:
        with (
            tc.tile_pool(name="const", bufs=1) as cpool,
            tc.tile_pool(name="wts", bufs=1) as wts,
            tc.tile_pool(name="kv", bufs=1) as kvp,
            tc.tile_pool(name="hsp", bufs=2) as hsp,
            tc.tile_pool(name="tabp", bufs=2) as tabp,
            tc.tile_pool(name="qnp", bufs=3) as qnp,
            tc.tile_pool(name="attp", bufs=3) as attp,
            tc.tile_pool(name="ptmp", bufs=4) as ptmp,
            tc.tile_pool(name="probs", bufs=24) as probs,
            tc.tile_pool(name="paccp", bufs=4) as paccp,
            tc.tile_pool(name="ysbp", bufs=3) as ysbp,
            tc.tile_pool(name="ps_pp", bufs=2, space="PSUM") as ps_pp,
            tc.tile_pool(name="ps_s", bufs=2, space="PSUM") as ps_s,
            tc.tile_pool(name="ps_ad", bufs=2, space="PSUM") as ps_ad,
            tc.tile_pool(name="ps_y", bufs=2, space="PSUM") as ps_y,
        ):
            wq_sb = wts.tile([P, NHT, NQC * D], bf, tag="wq")
            hs0 = hsp.tile([P, NHT, SCP], bf, tag="hs", name="hs0")
            for qt in range(4):
                h0, h1 = 4 * qt, 4 * (qt + 1)
                nc.sync.dma_start(out=wq_sb[:, h0:h1, :],
                                  in_=wqd[:, h0 * NQC * D:h1 * NQC * D])
                nc.scalar.dma_start(out=hs0[:, h0:h1, :],
                                    in_=hsd[:, 0, h0 * SCP:h1 * SCP])
            csts = cpool.tile([P, 6, P], bf, tag="csts")
            nc.sync.dma_start(out=csts[:], in_=cstd[:])
            ones_sb = csts[:, 0, :]
            dm_sb = csts[:, 2, :]
            em_sb = csts[:, 3, :]
            rq_sb = csts[:, 4, :]
            rk_sb = csts[:, 5, :]
            eps_sb = cpool.tile([P, 1], f32, tag="eps")
            nc.vector.memset(eps_sb[:], EPS)
            # preload the natural_log_exp table set: it covers every
            # activation used below (square/ln/exp/copy), so the table-load
            # pass never needs to swap tables mid-kernel
            nc.scalar.add_instruction(mybir.InstLoadActFuncSet(
                name=nc.get_next_instruction_name(), ins=[], outs=[],
                act_func_set_id=6))
            tab0 = tabp.tile([P, 4, SCP], bf, tag="tab", name="tab0")
            nc.sync.dma_start(out=tab0[:], in_=tabd[:, 0, :])
            wk_sb = wts.tile([P, NHT, NKVC * D], bf, tag="wk")
            nc.sync.dma_start(out=wk_sb[:], in_=wkd[:])
            wv_sb = wts.tile([P, NHT, NKVC * D], bf, tag="wv")
            nc.sync.dma_start(out=wv_sb[:], in_=wvd[:])
            wo_sb = wts.tile([P, NQC, H], bf, tag="wo")
            nc.sync.dma_start(out=wo_sb[:], in_=wod[:])

            kn_sb = kvp.tile([P, NKVC, S], bf, tag="kn")      # 1 MB
            v_sb = kvp.tile([P, NST, NKVC * D], bf, tag="v")  # 1 MB

            qn_tiles = [None] * NCH
            attn_tiles = [None] * NCH

            def proj(sc, defer_v=False):
                ve = nc.vector if sc == 0 else nc.gpsimd
                vf = nc.vector if sc == 0 else nc.gpsimd
                s0 = sc * SCP
                if sc == 0:
                    hs, tab = hs0, tab0
                else:
                    hs = hsp.tile([P, NHT, SCP], bf, tag="hs")
                    nc.sync.dma_start(out=hs[:], in_=hsd[:, sc, :])
                    tab = tabp.tile([P, 4, SCP], bf, tag="tab")
                    nc.sync.dma_start(out=tab[:], in_=tabd[:, sc, :])
                qn = qnp.tile([P, NQC, SCP], bf, tag="qn")
                qn_tiles[sc] = qn

                # head pairs: q(0,1), q(2,3), k(0,1) — [128, 2, 256] PSUM tiles.
                # RMS-norm is deferred: rope runs on the raw projection (the
                # per-position rsqrt commutes with the d-rotation), and the
                # chunk's Act sequence is [square,ln]x3 then exps, so the
                # activation table set switches only twice per chunk.
                # stage A: projections + raw copies (pp dies fast so the pp
                # pool never blocks PE); GPSIMD can't touch PSUM, so all
                # PSUM-reading elementwise work is on DVE/Act and Pool gets
                # the SBUF-only bf16 rope/normalize ops.
                pairs = (("q", 0), ("q", 2), ("k", 0))
                sqs, xnrs = [], []
                lnvs, qnraws = [None] * 3, [None] * 3
                for kind, m0 in pairs:
                    w_sb = wq_sb if kind == "q" else wk_sb
                    pp = ps_pp.tile([P, 2, SCP], f32, tag="pp")
                    for ht in range(NHT):
                        for mi in range(2):
                            m = m0 + mi
                            # one start per PSUM row: it marks the whole 2KB
                            # row pending-zero, so (ht0, mi1) must NOT restart
                            # or (ht1, mi0) would replace instead of accumulate
                            nc.tensor.matmul(
                                pp[:, mi, :], w_sb[:, ht, m * D:(m + 1) * D], hs[:, ht, :],
                                start=(ht == 0 and mi == 0), stop=(ht == NHT - 1),
                                skip_group_check=True)
                    sq = ptmp.tile([P, 2, SCP], bf, tag="sq")
                    nc.scalar.square(sq[:], pp[:])
                    xnr = ptmp.tile([P, 2, SCP], bf, tag="xnr")
                    nc.vector.tensor_copy(xnr[:], pp[:])
                    sqs.append(sq)
                    xnrs.append(xnr)
                # stage B: variance + rotation matmuls, ln, rope elementwise
                # (k pair first so kn is ready earliest for the next braid)
                border = (0, 1, 2)
                for pi in border:
                    kind, m0 = pairs[pi]
                    rot_sb = rq_sb if kind == "q" else rk_sb
                    cos_t = tab[:, 0, :] if kind == "q" else tab[:, 2, :]
                    sin_t = tab[:, 1, :] if kind == "q" else tab[:, 3, :]
                    vb = ps_ad.tile([P, 2, SCP], f32, tag="ad", name="vb")
                    nc.tensor.matmul(vb[:], ones_sb, sqs[pi][:], start=True, stop=True)
                    lnv = ptmp.tile([P, 2, SCP], f32, tag="lnv")
                    nc.scalar.activation(lnv[:], vb[:], AFT.Ln, bias=eps_sb[:], scale=1.0 / D)
                    lnvs[pi] = lnv
                    tcs = ptmp.tile([P, 2, SCP], bf, tag="tcs")
                    for mi in range(2):
                        ve.tensor_mul(tcs[:, mi, :], xnrs[pi][:, mi, :], cos_t)
                    # rotate_half via partition-offset reads: sin table is
                    # host-shuffled (sign + shifted norm weight folded in)
                    hh = D // 2
                    tsn = ptmp.tile([P, 2, SCP], bf, tag="tsn")
                    for mi in range(2):
                        nc.vector.tensor_mul(tsn[:hh, mi, :], xnrs[pi][hh:, mi, :], sin_t[hh:])
                        nc.vector.tensor_mul(tsn[hh:, mi, :], xnrs[pi][:hh, mi, :], sin_t[:hh])
                    qnr = ptmp.tile([P, 2, SCP], bf, tag="qnr")
                    ve.tensor_add(qnr[:], tcs[:], tsn[:])
                    qnraws[pi] = qnr
                # rsqrt = exp(-0.5*ln(var+eps)), then the deferred normalize
                for pi in border:
                    kind, m0 = pairs[pi]
                    sdi = ptmp.tile([P, 2, SCP], bf, tag="sdi")
                    nc.scalar.activation(sdi[:], lnvs[pi][:], AFT.Exp, scale=-0.5)
                    dst = qn[:, m0:m0 + 2, :] if kind == "q" else kn_sb[:, 0:2, s0:s0 + SCP]
                    vf.tensor_mul(dst, qnraws[pi][:], sdi[:])

                def vproj(ss):
                    st = sc * (SCP // P) + ss
                    pv = ps_y.tile([P, 2, SCP], f32, tag="y", name="pv")
                    for ht in range(NHT):
                        nc.tensor.matmul(
                            pv[:, 0, :], hs[:, ht, ss * P:(ss + 1) * P], wv_sb[:, ht, :],
                            start=(ht == 0), stop=(ht == NHT - 1))
                    nc.vector.tensor_copy(v_sb[:, st, :], pv[:, 0, :])
                if defer_v:
                    return [lambda: vproj(0), lambda: vproj(1)]
                vproj(0)
                vproj(1)
                return []

            def attn_scores(qc, h):
                segs = _segments(qc)
                qn = qn_tiles[qc]
                if h == 0:
                    attn_tiles[qc] = attp.tile([P, NQC, SCP], bf, tag="attn",
                                               name="attn")
                if True:
                    kvh = h // 2
                    # group segments into <=512-wide PSUM score tiles
                    groups, cur, cw = [], [], 0
                    for sg in segs:
                        if cw + sg[2] > 512:
                            groups.append(cur)
                            cur, cw = [], 0
                        cur.append((sg, cw))
                        cw += sg[2]
                    if cur:
                        groups.append(cur)
                    ptile = {}   # (t, c0) -> (p tile, offset)
                    for grp in groups:
                        gw = grp[-1][1] + grp[-1][0][2]
                        s_ps = ps_s.tile([P, 512], f32, tag="s")
                        for (t, c0, w, mk), off in grp:
                            nc.tensor.matmul(
                                s_ps[:, off:off + w],
                                kn_sb[:, kvh, t * P:(t + 1) * P],
                                qn[:, h, c0:c0 + w],
                                start=True, stop=True,
                                skip_group_check=True)
                        p = probs.tile([P, 512], bf, tag="p")
                        nc.scalar.activation(p[:, :gw], s_ps[:, :gw], AFT.Exp)
                        for (t, c0, w, mk), off in grp:
                            ptile[(t, c0)] = (p, off)
                            if mk in ("dm0", "dm"):
                                nc.vector.tensor_mul(p[:, off:off + P], p[:, off:off + P], dm_sb)
                            elif mk == "em":
                                nc.vector.tensor_mul(p[:, off:off + P], p[:, off:off + P], em_sb)
                            elif mk == "em1":
                                nc.vector.tensor_mul(p[:, off + P:off + 2 * P], p[:, off + P:off + 2 * P], em_sb)
                return segs, ptile

            def attn_pv(qc, h, segs, ptile):
                at = attn_tiles[qc]
                kvh = h // 2
                if True:
                    order = _pv_order(segs)
                    # prob-sum accumulation on DVE (bf16, 2x mode)
                    pacc = paccp.tile([P, SCP], bf, tag="pacc")
                    started = [False, False]
                    for (t, c0, w, mk) in order:
                        p, off = ptile[(t, c0)]
                        src = p[:, off:off + w]
                        halves = [0, 1] if w == 2 * P else [c0 // P]
                        fresh = [hh for hh in halves if not started[hh]]
                        if len(fresh) == len(halves):
                            nc.vector.tensor_copy(pacc[:, c0:c0 + w], src)
                        elif not fresh:
                            nc.vector.tensor_add(pacc[:, c0:c0 + w], pacc[:, c0:c0 + w], src)
                        else:
                            for hh in halves:
                                dsl = pacc[:, hh * P:(hh + 1) * P]
                                ssl = p[:, off + hh * P - c0:off + (hh + 1) * P - c0]
                                if started[hh]:
                                    nc.vector.tensor_add(dsl, dsl, ssl)
                                else:
                                    nc.vector.tensor_copy(dsl, ssl)
                        for hh in halves:
                            started[hh] = True

                    # PV accumulation (cols 0:256) and denominator (256:512)
                    ad_ps = ps_ad.tile([P, 2 * SCP], f32, tag="ad", name="a")
                    a_ps = ad_ps[:, :SCP]
                    for i, (t, c0, w, mk) in enumerate(order):
                        p, off = ptile[(t, c0)]
                        nc.tensor.matmul(
                            ad_ps[:, c0:c0 + w],
                            v_sb[:, t, kvh * D:(kvh + 1) * D],
                            p[:, off:off + w],
                            start=(i == 0), stop=(i == len(order) - 1),
                            skip_group_check=True)

                    d_ps = ad_ps[:, SCP:]
                    nc.tensor.matmul(d_ps, ones_sb, pacc[:], start=True, stop=True)
                    inv = ptmp.tile([P, SCP], f32, tag="inv")
                    nc.vector.reciprocal(inv[:], d_ps)
                    nc.vector.tensor_mul(at[:, h, :], a_ps, inv[:])

            def attn_head(qc, h):
                attn_pv(qc, h, *attn_scores(qc, h))


            def outproj_pairs(qc):
                """8 emit-closures, one per pair of output row-tiles."""
                state = {}

                def mk(pi):
                    def emit():
                        at = attn_tiles[qc]
                        mog, j = divmod(pi, 4)
                        if j == 0:
                            state[mog] = ysbp.tile([P, 8, SCP], bf, tag="ysb",
                                                   name="ysb")
                        y_sb = state[mog]
                        y_ps = ps_y.tile([P, 2, SCP], f32, tag="y")
                        for mi in range(2):
                            mo = 8 * mog + 2 * j + mi
                            for h in range(NQC):
                                nc.tensor.matmul(
                                    y_ps[:, mi, :], wo_sb[:, h, mo * P:(mo + 1) * P],
                                    at[:, h, :],
                                    start=(h == 0), stop=(h == NQC - 1))
                        if (qc * 8 + pi) % 2 == 0:
                            nc.vector.tensor_copy(y_sb[:, 2 * j:2 * j + 2, :], y_ps[:])
                        else:
                            nc.scalar.copy(y_sb[:, 2 * j:2 * j + 2, :], y_ps[:])
                        if j == 3:
                            nc.sync.dma_start(
                                out=yTd[:, 8 * mog:8 * (mog + 1), qc * SCP:(qc + 1) * SCP],
                                in_=y_sb[:])
                    return emit
                return [mk(pi) for pi in range(8)]

            def braid(aqc, oqc, vops=()):
                ops = list(vops) + (outproj_pairs(oqc) if oqc is not None else [])
                for h in range(NQC):
                    if aqc is not None:
                        attn_head(aqc, h)
                    for k in range(2):
                        if ops:
                            ops.pop(0)()
                while ops:
                    ops.pop(0)()

            for sc in range(NCH):
                vops = proj(sc, defer_v=(sc == 1))
                if sc >= 1:
                    braid(sc - 1, sc - 2 if sc >= 2 else None, vops)
            # tail: all 4 heads' scores first (exps stream on Act while PE
            # runs outproj(6)), then the PV/denominator passes, then outproj(7)
            ops6 = outproj_pairs(NCH - 2)
            st = []
            for h in range(NQC):
                st.append(attn_scores(NCH - 1, h))
                ops6.pop(0)()
            for h in range(NQC):
                attn_pv(NCH - 1, h, *st[h])
                ops6.pop(0)()
            for op in outproj_pairs(NCH - 1):
                op()

    nc.compile()
    _CACHE["nc"] = nc
    return nc


def _host_inputs(hidden_states, wq, wk, wv, wo, q_norm_weight, k_norm_weight):
    """Per-core input dicts (8 cores: c = 4*b + g)."""
    bf = ml_dtypes.bfloat16
    f = np.float32
    scale = 1.0 / math.sqrt(D)
    inv_freq = 1.0 / (THETA ** (np.arange(0, D, 2, dtype=np.float64) / D))
    t = np.arange(S, dtype=np.float64)
    freqs = np.outer(t, inv_freq)
    emb = np.concatenate([freqs, freqs], axis=-1)          # [S, D]
    cosT = np.cos(emb).T.astype(f)                         # [D, S]
    sinT = np.sin(emb).T.astype(f)
    qw = (1.0 + q_norm_weight).astype(f)
    kw = (1.0 + k_norm_weight).astype(f)

    R = np.zeros((D, D), f)
    hh = D // 2
    for i in range(hh):
        R[i, i + hh] = -1.0
        R[i + hh, i] = 1.0
    rqT = np.ascontiguousarray((R * qw[None, :]).T)
    rkT = np.ascontiguousarray((R * kw[None, :]).T)

    hh = D // 2
    def shuf_sin(sin_t, w):
        # table aligned with the xnr partition it multiplies: entry p holds
        # the factor for tsn[(p+hh) % D] = rot(w*x)[...]*sin[...]
        out = np.empty_like(sin_t)
        out[:hh] = sin_t[hh:] * w[:hh, None]
        out[hh:] = -sin_t[:hh] * w[hh:, None]
        return out
    cosq = cosT * qw[:, None] * scale
    sinq = shuf_sin(sinT, qw) * scale
    cosk = cosT * kw[:, None]
    sink = shuf_sin(sinT, kw)
    # packed rope tables: [128, NCH, (cosq|sinq|cosk|sink) x 256]
    tabd = np.stack([cosq, sinq, cosk, sink])              # [4, D, S]
    tabd = tabd.reshape(4, P, NCH, SCP).transpose(1, 2, 0, 3)
    tabd = np.ascontiguousarray(tabd.reshape(P, NCH, 4 * SCP)).astype(bf)

    r = np.arange(P)[:, None]
    c = np.arange(P)[None, :]
    dmask = np.where(c >= r, 1.0, 0.0).astype(f)           # diag: allow q_col >= k_row
    emask = np.where(r > c, 1.0, 0.0).astype(f)            # edge: allow k_row > q_col
    cstd = np.stack([np.ones((P, P), f), np.eye(P, dtype=f), dmask, emask, rqT, rkT], axis=1)
    cstd = np.ascontiguousarray(cstd.reshape(P, 6 * P)).astype(bf)

    def pmajor(mat, nblk):
        """[nblk*128, C] -> [128, nblk*C] with block index inside free dim."""
        nb, c = mat.shape[0] // P, mat.shape[1]
        return np.ascontiguousarray(
            mat.reshape(nb, P, c).transpose(1, 0, 2).reshape(P, nb * c))

    hsd = []
    for b in range(B):
        hsT = hidden_states[b].T.astype(f)                 # [H, S]
        a = hsT.reshape(NHT, P, NCH, SCP).transpose(1, 2, 0, 3)
        hsd.append(np.ascontiguousarray(a.reshape(P, NCH, NHT * SCP)).astype(bf))

    in_maps = []
    for core in range(8):
        b, g = divmod(core, 4)
        in_maps.append({
            "hsd": hsd[b],
            "wqd": pmajor(wq[512 * g:512 * (g + 1), :].T.astype(f), NHT).astype(bf),
            "wkd": pmajor(wk[256 * g:256 * (g + 1), :].T.astype(f), NHT).astype(bf),
            "wvd": pmajor(wv[256 * g:256 * (g + 1), :].T.astype(f), NHT).astype(bf),
            "wod": pmajor(wo[:, 512 * g:512 * (g + 1)].T.astype(f), NQC).astype(bf),
            "tabd": tabd, "cstd": cstd,
        })
    return in_maps


def _postprocess(results):
    out = np.empty((B, S, H), np.float32)
    for b in range(B):
        acc = results[4 * b]["yT"].astype(np.float32)
        for g in range(1, 4):
            acc = acc + results[4 * b + g]["yT"].astype(np.float32)
        # acc: [128, NHT, S] p-major -> y[h=mo*128+p, s] -> out [S, H]
        out[b] = acc.transpose(1, 0, 2).reshape(H, S).T
    return out


def kernel(hidden_states, wq, wk, wv, wo, q_norm_weight, k_norm_weight):
    nc = _build_nc()
    in_maps = _host_inputs(hidden_states, wq, wk, wv, wo, q_norm_weight, k_norm_weight)
    res = run_bass_kernel_spmd(nc, in_maps, list(range(8)))
    return _postprocess(res.results)
